# revision 5
# baseline (speedup 1.0000x reference)
"""CNNMoE kernel for 8 Trainium2 NeuronCores.

Self-contained: accepts FULL inputs (x [16,3,256,256] + params pytree),
returns FULL output (out [16,1,256,256], l_aux scalar).

v0 strategy: single jax.jit program, batch-sharded over the 8 cores
(2 images/core). Every expert runs densely on local tokens so the only
cross-core traffic is BN batch-stats + l_aux means (GSPMD-inserted
all-reduces). Expert weights are replicated (they are small).
"""

import numpy as np
import jax
import jax.numpy as jnp
from jax import lax
from jax.sharding import Mesh, NamedSharding, PartitionSpec as PS
from functools import partial

E, K, P, CIN, EMB = 8, 2, 16, 3, 256
D = CIN * P * P
DN = ('NCHW', 'OIHW', 'NCHW')


def conv2d(x, w, b, stride=1, pad=1, groups=1):
    y = lax.conv_general_dilated(x, w, (stride, stride), [(pad, pad), (pad, pad)],
                                 feature_group_count=groups, dimension_numbers=DN)
    return y + b[None, :, None, None]


def convT(x, w, b):
    # ConvTranspose2d(k=4, s=2, p=1) decomposed into 4 stride-1 2x2 convs
    # (one per output parity phase) — avoids lhs_dilation, which the
    # neuron tensorizer rejects in full-graph mode.
    # out[2m+r, 2n+s] = sum_{a,b in 0..1} w[i,o, kh(r,a), kw(s,b)] * x[m+dh(r,a), n+dw(s,b)]
    # phase r=0: kh = 3-2a, input index m-1+a ; r=1: kh = 2-2a, input m+a
    B, Ci, H, W = x.shape
    Co = w.shape[1]
    ys = []
    for r in (0, 1):
        for s in (0, 1):
            kh = [3, 1] if r == 0 else [2, 0]
            kw = [3, 1] if s == 0 else [2, 0]
            # kernel [Co, Ci, 2, 2]: W[o,i,a,b] = w[i,o,kh[a],kw[b]]
            wk = w[:, :, kh, :][:, :, :, kw].transpose(1, 0, 2, 3)
            ph = (1, 0) if r == 0 else (0, 1)
            pw = (1, 0) if s == 0 else (0, 1)
            y = lax.conv_general_dilated(x, wk, (1, 1), [ph, pw],
                                         dimension_numbers=DN)
            ys.append(y)
    z = jnp.stack(ys, axis=-1).reshape(B, Co, H, W, 2, 2)
    out = z.transpose(0, 1, 2, 4, 3, 5).reshape(B, Co, 2 * H, 2 * W)
    return out + b[None, :, None, None]


def top2(logits):
    # manual top-2 over last axis (size E) — avoids lax.top_k/sort lowering
    i1 = jnp.argmax(logits, axis=-1)
    v1 = jnp.max(logits, axis=-1)
    iota = jnp.arange(logits.shape[-1])[None, :]
    masked = jnp.where(iota == i1[:, None], -jnp.inf, logits)
    i2 = jnp.argmax(masked, axis=-1)
    v2 = jnp.max(masked, axis=-1)
    return jnp.stack([v1, v2], -1), jnp.stack([i1, i2], -1)


def bn(x, g, be, eps=1e-5):
    mu = x.mean((0, 2, 3), keepdims=True)
    var = ((x - mu) ** 2).mean((0, 2, 3), keepdims=True)
    return g[None, :, None, None] * (x - mu) * lax.rsqrt(var + eps) + be[None, :, None, None]


def ds_block(x, dww, dwb, pww, pwb, stride):
    x = jax.nn.relu(conv2d(x, dww, dwb, stride=stride, pad=1, groups=x.shape[1]))
    return jax.nn.relu(conv2d(x, pww, pwb, stride=1, pad=0))


def expert_fwd(p, flat):
    n = flat.shape[0]
    h = flat.reshape(n, CIN, P, P)
    h = ds_block(h, p['dw1_w'], p['dw1_b'], p['pw1_w'], p['pw1_b'], 2)
    h = ds_block(h, p['dw2_w'], p['dw2_b'], p['pw2_w'], p['pw2_b'], 2)
    h = ds_block(h, p['dw3_w'], p['dw3_b'], p['pw3_w'], p['pw3_b'], 2)
    feats = h.mean((2, 3))
    return feats @ p['th_w'].T + p['th_b'] + flat @ p['res_w'].T + p['res_b']


def forward(x, params):
    p = params
    B, C, H, W = x.shape
    hp, wp = H // P, W // P
    patches = x.reshape(B, C, hp, P, wp, P).transpose(0, 2, 4, 1, 3, 5).reshape(B * hp * wp, D)
    logits = patches @ p['gate_w'] + p['gate_b']
    vals, idx = top2(logits)
    gsc = jax.nn.softmax(vals, axis=-1)
    all_out = jax.vmap(expert_fwd, in_axes=(0, None))(p['experts'], patches)
    t = jnp.arange(patches.shape[0])
    moe_out = gsc[:, 0, None] * all_out[idx[:, 0], t] + gsc[:, 1, None] * all_out[idx[:, 1], t]
    probs = jax.nn.softmax(logits, axis=-1)
    ce = jax.nn.one_hot(idx[:, 0], E, dtype=x.dtype).mean(0)
    l_aux = E * jnp.sum(probs.mean(0) * ce)
    emb = moe_out.reshape(B, hp * wp, D) @ p['to_emb_w'].T + p['to_emb_b']
    feats = emb.transpose(0, 2, 1).reshape(B, EMB, hp, wp)
    s = jax.nn.relu(bn(conv2d(x, p['skip_w'], p['skip_b']), p['skip_g'], p['skip_be']))
    s = s.reshape(B, s.shape[1], hp, H // hp, wp, W // wp).mean((3, 5))
    h = jnp.concatenate([feats, s], axis=1)
    h = jax.nn.relu(bn(conv2d(h, p['dp1_w'], p['dp1_b']), p['dp1_g'], p['dp1_be']))
    h = jax.nn.relu(bn(conv2d(h, p['dp2_w'], p['dp2_b']), p['dp2_g'], p['dp2_be']))
    for u in ('up1', 'up2', 'up3', 'up4'):
        h = jax.nn.relu(bn(convT(h, p[u + '_w'], p[u + '_b']), p[u + '_g'], p[u + '_be']))
    h = jax.nn.relu(bn(conv2d(h, p['fc1_w'], p['fc1_b']), p['fc1_g'], p['fc1_be']))
    # fc2 is a 1x1 conv to a single channel — express as einsum (the neuron
    # tensorizer's conv transform rejects Cout=1).
    out = jnp.einsum('bchw,oc->bohw', h, p['fc2_w'][:, :, 0, 0]) + p['fc2_b'][None, :, None, None]
    return out, l_aux


_cache = {}


def _get_fn():
    if 'fn' in _cache:
        return _cache['fn']
    devs = [d for d in jax.devices() if d.platform != 'cpu'][:8]
    if len(devs) < 8:
        devs = jax.devices()[:8]
    mesh = Mesh(np.array(devs), ('b',))
    xsh = NamedSharding(mesh, PS('b'))
    rep = NamedSharding(mesh, PS())
    fn = jax.jit(
        forward,
        in_shardings=(xsh, rep),
        out_shardings=(xsh, rep),
    )
    _cache['fn'] = (fn, xsh, rep)
    return _cache['fn']


def kernel(x, params):
    fn, xsh, rep = _get_fn()
    x = jax.device_put(jnp.asarray(np.asarray(x), jnp.float32), xsh)
    params = jax.tree.map(lambda a: jax.device_put(jnp.asarray(np.asarray(a), jnp.float32), rep), params)
    out, laux = fn(x, params)
    return np.asarray(out), np.asarray(laux)
